# revision 1
# baseline (speedup 1.0000x reference)
"""BertCorrector kernel for 8 TRN2 NeuronCores.

Computes: segment-mean merge of subword encodings (sorted per-row segment
ids) followed by a dense vocab projection:
    merged[b,w,:] = mean_{s: ids[b,s]==w} enc[b,s,:]   (0 if empty)
    logits = merged @ W + b

Strategy: data-parallel over batch (4 samples/core).  The segment-mean is
computed on the TensorEngine as enc^T @ S where S is a per-sample one-hot
matrix pre-scaled by 1/count (built host-side from segment_ids).  That
directly yields merged TRANSPOSED ([H, W] chunks), which is exactly the
stationary-operand layout the vocab-projection matmul needs.  All matmul
inputs are bf16 (fp32 PSUM accumulation); the output is written f32.
"""

import numpy as np
import ml_dtypes

B, S, H = 32, 512, 768
V = 8192
WMAX = 256
NCORES = 8
PB = B // NCORES  # samples per core
P = 128

KC = S // P   # 4 token chunks (contraction of stage A)
KO = H // P   # 6 hidden chunks
WT = WMAX // P  # 2 word tiles
NV = 512      # vocab tile
NT = V // NV  # 16 vocab tiles

_compiled = None


def _build_program():
    import concourse.bass as bass
    import concourse.mybir as mybir
    from concourse import bacc
    from concourse.tile import TileContext

    bf16 = mybir.dt.bfloat16
    f32 = mybir.dt.float32

    nc = bacc.Bacc()
    enc_d = nc.dram_tensor("enc", [PB, S, H], bf16, kind="ExternalInput")
    oneh_d = nc.dram_tensor("oneh", [PB, S, WMAX], bf16, kind="ExternalInput")
    w_d = nc.dram_tensor("wmat", [H, V], bf16, kind="ExternalInput")
    out_d = nc.dram_tensor("out", [PB, WMAX, V], f32, kind="ExternalOutput")

    enc_r = enc_d.rearrange("b (kc p) h -> b p kc h", p=P)
    oneh_r = oneh_d.rearrange("b (kc p) w -> b p kc w", p=P)
    w_r = w_d.rearrange("(ko p) v -> p ko v", p=P)

    with TileContext(nc) as tc:
        with (
            tc.tile_pool(name="persist", bufs=1) as persist,
            tc.tile_pool(name="encp", bufs=2) as encp,
            tc.tile_pool(name="onehp", bufs=2) as onehp,
            tc.tile_pool(name="wp", bufs=3) as wp,
            tc.tile_pool(name="outp", bufs=8) as outp,
            tc.tile_pool(name="ps1", bufs=2, space="PSUM") as ps1,
            tc.tile_pool(name="ps2", bufs=6, space="PSUM") as ps2,
        ):
            # mergedT[h_in_chunk, ko, s, w] resident in SBUF (bf16)
            mergedT = persist.tile([P, KO, PB, WMAX], bf16)

            # ---- Stage A: mergedT = enc^T @ scaled_onehot, per sample ----
            for s in range(PB):
                enc_sb = encp.tile([P, KC, H], bf16, tag="enc")
                nc.sync.dma_start(out=enc_sb[:], in_=enc_r[s])
                oneh_sb = onehp.tile([P, KC, WMAX], bf16, tag="oneh")
                nc.sync.dma_start(out=oneh_sb[:], in_=oneh_r[s])
                for ko in range(KO):
                    pt = ps1.tile([P, WMAX], f32, tag="ps1")
                    for kc in range(KC):
                        nc.tensor.matmul(
                            pt[:],
                            lhsT=enc_sb[:, kc, ko * P:(ko + 1) * P],
                            rhs=oneh_sb[:, kc, :],
                            start=(kc == 0),
                            stop=(kc == KC - 1),
                        )
                    nc.any.tensor_copy(out=mergedT[:, ko, s, :], in_=pt[:])

            # ---- Stage B: out[s, w, v] = mergedT^T @ W, tiled over vocab ----
            for n in range(NT):
                w_sb = wp.tile([P, KO, NV], bf16, tag="w")
                nc.sync.dma_start(out=w_sb[:], in_=w_r[:, :, n * NV:(n + 1) * NV])
                for s in range(PB):
                    for wt in range(WT):
                        pt = ps2.tile([P, NV], f32, tag="ps2")
                        for ko in range(KO):
                            nc.tensor.matmul(
                                pt[:],
                                lhsT=mergedT[:, ko, s, wt * P:(wt + 1) * P],
                                rhs=w_sb[:, ko, :],
                                start=(ko == 0),
                                stop=(ko == KO - 1),
                            )
                        ot = outp.tile([P, NV], f32, tag="out")
                        nc.any.tensor_copy(out=ot[:], in_=pt[:])
                        nc.sync.dma_start(
                            out=out_d[s, wt * P:(wt + 1) * P, n * NV:(n + 1) * NV],
                            in_=ot[:],
                        )

    nc.finalize()
    return nc


def _get_program():
    global _compiled
    if _compiled is None:
        _compiled = _build_program()
    return _compiled


def _prep_inputs(bert_encodings, segment_ids, W):
    enc_bf = np.asarray(bert_encodings, dtype=np.float32).astype(ml_dtypes.bfloat16)
    w_bf = np.asarray(W, dtype=np.float32).astype(ml_dtypes.bfloat16)

    ids = np.asarray(segment_ids).astype(np.int64)
    flat = (ids + np.arange(B, dtype=np.int64)[:, None] * WMAX).ravel()
    counts = np.bincount(flat, minlength=B * WMAX).reshape(B, WMAX)
    inv = (1.0 / np.maximum(counts, 1)).astype(np.float32)

    oneh = np.zeros((B, S, WMAX), dtype=ml_dtypes.bfloat16)
    bidx = np.repeat(np.arange(B), S)
    sidx = np.tile(np.arange(S), B)
    widx = ids.ravel()
    oneh[bidx, sidx, widx] = inv[bidx, widx].astype(ml_dtypes.bfloat16)
    return enc_bf, w_bf, oneh


def kernel(bert_encodings, segment_ids, W, b, num_words, _trace=False):
    from concourse.bass_utils import run_bass_kernel_spmd

    assert int(num_words) == WMAX
    enc_bf, w_bf, oneh = _prep_inputs(bert_encodings, segment_ids, W)

    nc = _get_program()
    core_ids = list(range(NCORES))
    in_maps = [
        {
            "enc": enc_bf[c * PB:(c + 1) * PB],
            "oneh": oneh[c * PB:(c + 1) * PB],
            "wmat": w_bf,
        }
        for c in core_ids
    ]
    res = run_bass_kernel_spmd(nc, in_maps, core_ids, trace=_trace)
    out = np.concatenate([res.results[c]["out"] for c in core_ids], axis=0)
    out = np.ascontiguousarray(out.reshape(B, WMAX, V))

    bias = np.asarray(b, dtype=np.float32)
    if np.any(bias):
        out = out + bias

    if _trace:
        kernel._last_exec_time_ns = res.exec_time_ns
    return out
